# revision 35
# baseline (speedup 1.0000x reference)
"""Trainium2 Bass kernel for EqualizedModConv2d (StyleGAN2 modulated conv).

Math (per sample b):
  s[i]  = (y @ dense_w.T) * LATENT**-0.5 + dense_b                 # style
  ww    = weight * W_MUL * s[i]                                    # modulate
  d[o]  = rsqrt(sum_{i,k}(ww^2) + 1e-8)                            # demodulate
  out   = conv2d(x, ww * d, VALID)

Strategy: Winograd F(2x2, 3x3) — 2.25x fewer PE MACs than direct conv.
  U = G w G^T            (host, bf16)           [cout, cin, 4, 4]
  V = B^T (s*x) B        (on-device transforms) [cin, 4, 4, 31, 31]
  m[i][j] = sum_cin U[i][j] (x) V[i][j]         (PE, bf16 matmuls, fp32 PSUM)
  out = (A^T m A) * D    (demod folded into the final ACT interleave)

Demod: D[b,o] = 1/sqrt(T + 1e-8/W_MUL^2), T = (s*s) @ wsq.T, wsq host fp32.

Distribution: data-parallel over batch, 2 samples per core on 8 cores.
Per core: 2048 accumulating bf16 matmuls (2 samples x 4 row-groups x
4 cout-tiles x 4 j-cols x [4 i-rows x 4 cin-chunks]), each N=31*ntr cols.
HW notes baked in: scalar_tensor_tensor runs 2 cycles/elem on DVE (avoid);
plain tensor_tensor bf16 stride-1 hits the 2x mode; GPSIMD cannot touch
PSUM and its software loops are ~10x slow on strided data (avoid entirely);
ACT pays ~290ns access per op, so ops are merged as large as AP dims allow.
Engine split: ACT = style-scale/eo-split + most PSUM quad drains + final
demod-scale interleave; DVE = winograd transforms + inverse combines
(merged per cout-tile) + a few direct-from-PSUM quad combines.
"""

import sys
import types

import numpy as np

B, CIN, COUT, LATENT = 16, 512, 512, 512
H = W = 64
KH = KW = 3
OH = OW = 62
N_CORES = 8
BL = B // N_CORES  # samples per core
IC = CIN // 128    # cin chunks
OC = COUT // 128   # cout tiles
MUL_DENSE = float(LATENT ** -0.5)
# rsqrt(W_MUL^2 * T + 1e-8) * W_MUL == rsqrt(T + 1e-8 / W_MUL^2)
EPS2 = float(1e-8 * (CIN * KH * KW))
# winograd tile-row groups: (tile-row start, tile-row count)
GROUPS = [(0, 8), (8, 8), (16, 8), (24, 7)]
NTC = 31  # tile cols
N_VCT = 0   # how many (of 32) cout-tile rounds drain on DVE instead of ACT
GPS_Q = 0   # of every 4 cout-tile rounds, how many route q-intermediates to GPSIMD

_cache = {}


def _ensure_ntff_hook():
    """The agent image's antenv lacks axon_hooks, so boot silently skipped NTFF
    hook registration; recreate the module + hook so trace=True works."""
    if "antenv.axon_hooks" in sys.modules:
        return
    try:
        import antenv
        from trn_agent_boot.trn_boot import _ntff_profile_via_ctypes
    except ImportError:
        return
    mod = types.ModuleType("antenv.axon_hooks")
    mod._hook = None

    def _set(h):
        mod._hook = h

    def _get():
        return mod._hook

    mod.set_axon_ntff_profile_hook = _set
    mod.get_axon_ntff_profile_hook = _get
    sys.modules["antenv.axon_hooks"] = mod
    antenv.axon_hooks = mod
    try:
        _set(_ntff_profile_via_ctypes("/opt/axon/libaxon_pjrt.so"))
    except OSError:
        pass


def build():
    """Emit + compile the per-core Tile program. Returns the Bass object."""
    import concourse.bass as bass
    import concourse.bacc as bacc
    import concourse.tile as tile
    from concourse import mybir

    f32 = mybir.dt.float32
    bf16 = mybir.dt.bfloat16
    AF = mybir.ActivationFunctionType
    PSUM = bass.MemorySpace.PSUM

    nc = bacc.Bacc("TRN2", target_bir_lowering=False, debug=False)

    x_d = nc.dram_tensor("x", [BL, CIN, H, W], f32, kind="ExternalInput")
    # winograd weights, host-transformed: [ct, p(cin%128), j, ic, i, o(128)]
    uw_d = nc.dram_tensor("uw", [OC, 128, 4, IC, 4, 128], bf16, kind="ExternalInput")
    dwt_d = nc.dram_tensor("dwt", [LATENT, CIN], f32, kind="ExternalInput")    # [l,c]
    yt_d = nc.dram_tensor("yt", [LATENT, BL], f32, kind="ExternalInput")       # [l,b]
    db_d = nc.dram_tensor("db", [CIN, 1], f32, kind="ExternalInput")
    wsq_d = nc.dram_tensor("wsq", [IC, 128, COUT], f32, kind="ExternalInput")  # [ic,p,o]
    out_d = nc.dram_tensor("out", [BL, COUT, OH, OW], bf16, kind="ExternalOutput")

    with tile.TileContext(nc) as tc:
        with (
            tc.tile_pool(name="persist", bufs=1) as persist,
            tc.tile_pool(name="small", bufs=1) as small,
            tc.tile_pool(name="xstage", bufs=3) as xstage,
            tc.tile_pool(name="xeo", bufs=2) as xeo,
            tc.tile_pool(name="tpool", bufs=2) as tpool,
            tc.tile_pool(name="vpool", bufs=2) as vpool,
            tc.tile_pool(name="mpool", bufs=2) as mpool,
            tc.tile_pool(name="qpool", bufs=2) as qpool,
            tc.tile_pool(name="npool", bufs=2) as npool,
            tc.tile_pool(name="opool", bufs=2) as opool,
            tc.tile_pool(name="psq", bufs=2, space=PSUM) as psq,
        ):
            # ---- persistent winograd weights ----
            ubf = persist.tile([128, OC, 4, IC, 4, 128], bf16)

            # ---- param DMAs (style params first: they gate everything) ----
            dwt_sb = small.tile([128, 4, CIN], f32)
            nc.sync.dma_start(dwt_sb[:, :, :], dwt_d.ap().rearrange("(l p) c -> p l c", p=128))
            yt_sb = small.tile([128, 4, BL], f32)
            nc.sync.dma_start(yt_sb[:, :, :], yt_d.ap().rearrange("(l p) b -> p l b", p=128))
            db_sb = small.tile([128, 4, 1], f32)
            nc.sync.dma_start(db_sb[:, :, :], db_d.ap().rearrange("(c p) u -> p c u", p=128))
            wsq_sb = small.tile([128, IC, COUT], f32)
            for ic in range(IC):
                nc.sync.dma_start(wsq_sb[:, ic, :], wsq_d[ic])

            # ---- x (b0, g0) chunk 0 + U ct0, then the rest ----
            xq_first = []
            for ic in range(IC):
                xr = xstage.tile([128, 18, 64], f32, tag="xr")
                nc.sync.dma_start(xr[:, 0:18, :], x_d[0, ic * 128:(ic + 1) * 128, 0:18, :])
                xq_first.append(xr)
                if ic == 0:
                    nc.sync.dma_start(ubf[:, 0], uw_d[0])

            # ---- remaining U tiles ----
            for ct in range(1, OC):
                nc.sync.dma_start(ubf[:, ct], uw_d[ct])

            # ---- style: s = (y @ dense_w.T) * mul + b (ssq deferred so the
            #      ACT queue reaches the eo copies as early as possible) ----
            s_sb = small.tile([128, IC, BL], f32)
            ssq = small.tile([128, IC, BL], f32)
            style_ps = []
            for ctc in range(IC):
                ps = psq.tile([128, 4, 512], f32, tag="pq")
                for lc in range(4):
                    nc.tensor.matmul(
                        ps[:, 0, 0:BL],
                        dwt_sb[:, lc, ctc * 128:(ctc + 1) * 128],
                        yt_sb[:, lc, :],
                        start=(lc == 0),
                        stop=(lc == 3),
                    )
                nc.scalar.activation(
                    s_sb[:, ctc, :], ps[:, 0, 0:BL], AF.Identity,
                    bias=db_sb[:, ctc, :], scale=MUL_DENSE,
                )

            # ---------------- transform emitters ----------------
            def emit_xdma(b, g, ic):
                r0, ntr = GROUPS[g]
                nrows = 2 * ntr + 2
                xr = xstage.tile([128, 18, 64], f32, tag="xr")
                nc.sync.dma_start(
                    xr[:, 0:nrows, :],
                    x_d[b, ic * 128:(ic + 1) * 128, 2 * r0:2 * r0 + nrows, :],
                )
                return xr

            def emit_eo(b, g, ic, xr):
                """style scale + even/odd column split + bf16 convert (ACT)."""
                r0, ntr = GROUPS[g]
                nrows = 2 * ntr + 2
                xe = xeo.tile([128, 2, 18, 32], bf16, tag="xe")
                nc.scalar.activation(
                    xe[:, 0, 0:nrows, :], xr[:, 0:nrows, 0:64:2], AF.Copy,
                    scale=s_sb[:, ic, b:b + 1])
                nc.scalar.activation(
                    xe[:, 1, 0:nrows, :], xr[:, 0:nrows, 1:64:2], AF.Copy,
                    scale=s_sb[:, ic, b:b + 1])
                return xe

            def emit_rowcol(b, g, ic, xe, vg, eng=None):
                """winograd row + col passes (bf16 stride-1 -> 2x mode).

                Row pass merges even/odd halves per combo; col pass merges the
                4 i-combos per j output. eng picks DVE (default) or GPSIMD."""
                eng = eng or nc.vector
                r0, ntr = GROUPS[g]
                n2 = 2 * ntr
                te = tpool.tile([128, 2, 4, 8, 32], bf16, tag="te")
                d = xe
                r_a = d[:, :, 0:n2:2, :]
                r_b = d[:, :, 1:n2 + 1:2, :]
                r_c = d[:, :, 2:n2 + 2:2, :]
                r_d = d[:, :, 3:n2 + 2:2, :]
                eng.tensor_sub(te[:, :, 0, 0:ntr, :], r_a, r_c)
                eng.tensor_add(te[:, :, 1, 0:ntr, :], r_b, r_c)
                eng.tensor_sub(te[:, :, 2, 0:ntr, :], r_c, r_b)
                eng.tensor_sub(te[:, :, 3, 0:ntr, :], r_b, r_d)
                tev = te[:, 0, :, 0:ntr, :]
                tov = te[:, 1, :, 0:ntr, :]
                c_a = tev[:, :, :, 0:31]
                c_b = tov[:, :, :, 0:31]
                c_c = tev[:, :, :, 1:32]
                c_d = tov[:, :, :, 1:32]
                eng.tensor_sub(vg[:, ic, 0, :, 0:ntr, :], c_a, c_c)
                eng.tensor_add(vg[:, ic, 1, :, 0:ntr, :], c_b, c_c)
                eng.tensor_sub(vg[:, ic, 2, :, 0:ntr, :], c_c, c_b)
                eng.tensor_sub(vg[:, ic, 3, :, 0:ntr, :], c_b, c_d)

            def new_vg():
                return vpool.tile([128, IC, 4, 4, 8, 31], bf16, tag="vg", name="vg")

            # ---- transforms for (b0, g0), plus a head start on (b0, g1):
            #      DVE is the steady-state constraint, so pre-pay its debt
            #      while the PE is still waiting on U/x DMAs ----
            vg_cur = new_vg()
            for ic in range(IC):
                xe0 = emit_eo(0, 0, ic, xq_first[ic])
                emit_rowcol(0, 0, ic, xe0, vg_cur)
            for ctc in range(IC):
                nc.scalar.activation(ssq[:, ctc, :], s_sb[:, ctc, :], AF.Square)
            vg_g1 = new_vg()
            for ic in range(2):
                xr1 = emit_xdma(0, 1, ic)
                xe1 = emit_eo(0, 1, ic, xr1)
                emit_rowcol(0, 1, ic, xe1, vg_g1)

            # ---- demod: T = wsq.T @ ssq ; D = 1/sqrt(T + eps') ----
            #      (emitted after the transforms so the ACT sqrt does not
            #       stall the in-order ACT queue ahead of the eo copies)
            sqrt_t = small.tile([128, OC, BL], f32)
            d_sb = small.tile([128, OC, BL], f32)
            eps_sb = small.tile([128, 1], f32)
            nc.gpsimd.memset(eps_sb[:, :], EPS2)
            for ot in range(OC):
                ps = psq.tile([128, 4, 512], f32, tag="pq")
                for ic in range(IC):
                    nc.tensor.matmul(
                        ps[:, 0, 0:BL],
                        wsq_sb[:, ic, ot * 128:(ot + 1) * 128],
                        ssq[:, ic, :],
                        start=(ic == 0),
                        stop=(ic == 3),
                    )
                nc.scalar.activation(
                    sqrt_t[:, ot, :], ps[:, 0, 0:BL], AF.Sqrt,
                    bias=eps_sb[:, :], scale=1.0,
                )
                nc.vector.reciprocal(d_sb[:, ot, :], sqrt_t[:, ot, :])

            # ---------------- inverse-transform helpers ----------------
            def drain(pq, ms, j, N, dsc, on_dve):
                """PSUM quad -> bf16 staging, demod scale folded in."""
                if on_dve:
                    nc.vector.tensor_scalar_mul(ms[:, j, :, 0:N], pq[:, :, 0:N], dsc)
                else:
                    nc.scalar.activation(
                        ms[:, j, :, 0:N], pq[:, :, 0:N], AF.Copy, scale=dsc)

            def combine_ms(ms, nb, N, qeng):
                """A^T row-combine over all 4 j at once (bf16, 2x mode)."""
                qn = qpool.tile([128, 2, 4, 248], bf16, tag="qc")
                qeng.tensor_add(qn[:, 0, :, 0:N], ms[:, :, 0, 0:N], ms[:, :, 1, 0:N])
                qeng.tensor_sub(qn[:, 1, :, 0:N], ms[:, :, 1, 0:N], ms[:, :, 2, 0:N])
                nc.vector.tensor_add(nb[:, 0, :, 0:N], qn[:, 0, :, 0:N], ms[:, :, 2, 0:N])
                nc.vector.tensor_sub(nb[:, 1, :, 0:N], qn[:, 1, :, 0:N], ms[:, :, 3, 0:N])

            def stage2(b, g, ct, nb, qeng):
                """A col-combine; finals write the interleaved bf16 output."""
                r0, ntr = GROUPS[g]
                N = ntr * NTC
                ob = opool.tile([128, 8, 2, 31, 2], bf16, tag="ob")
                q2 = qpool.tile([128, 2, 2, 248], bf16, tag="q2")
                qeng.tensor_add(
                    q2[:, 0, :, 0:N], nb[:, :, 0, 0:N], nb[:, :, 1, 0:N])
                qeng.tensor_sub(
                    q2[:, 1, :, 0:N], nb[:, :, 1, 0:N], nb[:, :, 2, 0:N])
                nc.vector.tensor_add(
                    ob[:, 0:ntr, :, :, 0].transpose([0, 2, 1, 3]),
                    q2[:, 0, :, 0:N], nb[:, :, 2, 0:N])
                nc.vector.tensor_sub(
                    ob[:, 0:ntr, :, :, 1].transpose([0, 2, 1, 3]),
                    q2[:, 1, :, 0:N], nb[:, :, 3, 0:N])
                nc.sync.dma_start(
                    out_d[b, ct * 128:(ct + 1) * 128, 2 * r0:2 * r0 + 2 * ntr, :],
                    ob[:, 0:ntr, :, :, :],
                )

            # ---------------- main loop over (sample, row-group) ----------------
            for gi in range(2 * 4):
                b, g = divmod(gi, 4)
                r0, ntr = GROUPS[g]
                N = ntr * NTC
                nxt = gi + 1 if gi + 1 < 8 else None
                nb_, ng = divmod(nxt, 4) if nxt is not None else (None, None)
                if gi == 0:
                    vg_next = vg_g1
                elif nxt is not None:
                    vg_next = new_vg()
                else:
                    vg_next = None
                xq_next = [None] * IC
                xe_next = [None] * IC
                for ct in range(OC):
                    vct = (gi * 4 + ct) >= (32 - N_VCT)
                    nb = npool.tile([128, 2, 4, 248], bf16, tag="nb")
                    ms = mpool.tile([128, 4, 4, 248], bf16, tag="ms", name="ms")
                    for j in range(4):
                        pq = psq.tile([128, 4, 512], f32, tag="pq")
                        for i in range(4):
                            for ic in range(IC):
                                nc.tensor.matmul(
                                    pq[:, i, 0:N],
                                    ubf[:, ct, j, ic, i, :],
                                    vg_cur[:, ic, j, i, 0:ntr, :],
                                    start=(ic == 0),
                                    stop=(ic == IC - 1),
                                )
                        drain(pq, ms, j, N, d_sb[:, ct, b:b + 1], vct)
                    qeng = nc.gpsimd if (ct % 4) < GPS_Q else nc.vector
                    combine_ms(ms, nb, N, qeng)
                    # prefetch next group's input pipeline (group 0: chunks 0,1
                    # of g1 were already done in the prologue)
                    if nxt is not None:
                        lo = 2 if gi == 0 else 0
                        if ct == 0:
                            for c in range(lo, 3):
                                xq_next[c] = emit_xdma(nb_, ng, c)
                            for c in range(lo, 2):
                                xe_next[c] = emit_eo(nb_, ng, c, xq_next[c])
                            if lo == 0:
                                emit_rowcol(nb_, ng, 0, xe_next[0], vg_next)
                        elif ct == 1:
                            xq_next[3] = emit_xdma(nb_, ng, 3)
                            xe_next[2] = emit_eo(nb_, ng, 2, xq_next[2])
                            if lo == 0:
                                emit_rowcol(nb_, ng, 1, xe_next[1], vg_next)
                        elif ct == 2:
                            xe_next[3] = emit_eo(nb_, ng, 3, xq_next[3])
                            emit_rowcol(nb_, ng, 2, xe_next[2], vg_next)
                        else:
                            emit_rowcol(nb_, ng, 3, xe_next[3], vg_next)
                    stage2(b, g, ct, nb, qeng)
                vg_cur = vg_next

    nc.compile()
    return nc


def run(inputs, profile=False):
    """inputs: dict with full-size x/y/dense_w/dense_b/weight numpy arrays.
    Returns (out [16,512,62,62] float32, exec_time_ns or None)."""
    import ml_dtypes
    from concourse.bass_utils import run_bass_kernel_spmd

    if "nc" not in _cache:
        _cache["nc"] = build()
    nc = _cache["nc"]

    x = np.ascontiguousarray(np.asarray(inputs["x"], dtype=np.float32))
    y = np.ascontiguousarray(np.asarray(inputs["y"], dtype=np.float32))
    dense_w = np.asarray(inputs["dense_w"], dtype=np.float32)
    dense_b = np.asarray(inputs["dense_b"], dtype=np.float32)
    weight = np.asarray(inputs["weight"], dtype=np.float32)

    # host-side winograd weight transform U = G w G^T -> [ct, p, j, ic, i, o] bf16
    G = np.array([[1, 0, 0], [.5, .5, .5], [.5, -.5, .5], [0, 0, 1]], np.float32)
    U = np.einsum("ak,oikl,bl->oiab", G, weight, G, optimize=True)  # [o, c, a, b]
    uw = np.ascontiguousarray(
        U.reshape(OC, 128, IC, 128, 4, 4).transpose(0, 3, 5, 2, 4, 1)
        .astype(ml_dtypes.bfloat16))
    # wsq[o,c] = sum_k w^2 (fp32, host) -> [ic, p, o]
    wsq = np.ascontiguousarray(
        (weight.astype(np.float64) ** 2).sum(axis=(2, 3)).astype(np.float32)
        .T.reshape(IC, 128, COUT))
    dwt = np.ascontiguousarray(dense_w.T)                          # [l, c]
    db = np.ascontiguousarray(dense_b.reshape(CIN, 1))

    in_maps = []
    for c in range(N_CORES):
        sl = slice(c * BL, (c + 1) * BL)
        in_maps.append({
            "x": x[sl],
            "uw": uw,
            "dwt": dwt,
            "yt": np.ascontiguousarray(y[sl].T),                   # [l, b]
            "db": db,
            "wsq": wsq,
        })

    if profile:
        _ensure_ntff_hook()
    res = run_bass_kernel_spmd(
        nc, in_maps, core_ids=list(range(N_CORES)), trace=profile)
    out = np.concatenate(
        [np.asarray(r["out"]).astype(np.float32) for r in res.results], axis=0)
    return out, res.exec_time_ns


def kernel(**inputs) -> np.ndarray:
    out, _ = run(inputs)
    return out


# revision 36
# speedup vs baseline: 1.0070x; 1.0070x over previous
"""Trainium2 Bass kernel for EqualizedModConv2d (StyleGAN2 modulated conv).

Math (per sample b):
  s[i]  = (y @ dense_w.T) * LATENT**-0.5 + dense_b                 # style
  ww    = weight * W_MUL * s[i]                                    # modulate
  d[o]  = rsqrt(sum_{i,k}(ww^2) + 1e-8)                            # demodulate
  out   = conv2d(x, ww * d, VALID)

Strategy: Winograd F(2x2, 3x3) — 2.25x fewer PE MACs than direct conv.
  U = G w G^T            (host, bf16)           [cout, cin, 4, 4]
  V = B^T (s*x) B        (on-device transforms) [cin, 4, 4, 31, 31]
  m[i][j] = sum_cin U[i][j] (x) V[i][j]         (PE, bf16 matmuls, fp32 PSUM)
  out = (A^T m A) * D    (demod folded into the final ACT interleave)

Demod: D[b,o] = 1/sqrt(T + 1e-8/W_MUL^2), T = (s*s) @ wsq.T, wsq host fp32.

Distribution: data-parallel over batch, 2 samples per core on 8 cores.
Per core: 2048 accumulating bf16 matmuls (2 samples x 4 row-groups x
4 cout-tiles x 4 j-cols x [4 i-rows x 4 cin-chunks]), each N=31*ntr cols.
HW notes baked in: scalar_tensor_tensor runs 2 cycles/elem on DVE (avoid);
plain tensor_tensor bf16 stride-1 hits the 2x mode; GPSIMD cannot touch
PSUM and its software loops are ~10x slow on strided data (avoid entirely);
ACT pays ~290ns access per op, so ops are merged as large as AP dims allow.
Engine split: ACT = style-scale/eo-split + most PSUM quad drains + final
demod-scale interleave; DVE = winograd transforms + inverse combines
(merged per cout-tile) + a few direct-from-PSUM quad combines.
"""

import sys
import types

import numpy as np

B, CIN, COUT, LATENT = 16, 512, 512, 512
H = W = 64
KH = KW = 3
OH = OW = 62
N_CORES = 8
BL = B // N_CORES  # samples per core
IC = CIN // 128    # cin chunks
OC = COUT // 128   # cout tiles
MUL_DENSE = float(LATENT ** -0.5)
# rsqrt(W_MUL^2 * T + 1e-8) * W_MUL == rsqrt(T + 1e-8 / W_MUL^2)
EPS2 = float(1e-8 * (CIN * KH * KW))
# winograd tile-row groups: (tile-row start, tile-row count)
GROUPS = [(0, 8), (8, 8), (16, 8), (24, 7)]
NTC = 31  # tile cols
N_VCT = 0   # how many (of 32) cout-tile rounds drain on DVE instead of ACT
GPS_Q = 0   # of every 4 cout-tile rounds, how many route q-intermediates to GPSIMD

_cache = {}


def _ensure_ntff_hook():
    """The agent image's antenv lacks axon_hooks, so boot silently skipped NTFF
    hook registration; recreate the module + hook so trace=True works."""
    if "antenv.axon_hooks" in sys.modules:
        return
    try:
        import antenv
        from trn_agent_boot.trn_boot import _ntff_profile_via_ctypes
    except ImportError:
        return
    mod = types.ModuleType("antenv.axon_hooks")
    mod._hook = None

    def _set(h):
        mod._hook = h

    def _get():
        return mod._hook

    mod.set_axon_ntff_profile_hook = _set
    mod.get_axon_ntff_profile_hook = _get
    sys.modules["antenv.axon_hooks"] = mod
    antenv.axon_hooks = mod
    try:
        _set(_ntff_profile_via_ctypes("/opt/axon/libaxon_pjrt.so"))
    except OSError:
        pass


def build():
    """Emit + compile the per-core Tile program. Returns the Bass object."""
    import concourse.bass as bass
    import concourse.bacc as bacc
    import concourse.tile as tile
    from concourse import mybir

    f32 = mybir.dt.float32
    bf16 = mybir.dt.bfloat16
    AF = mybir.ActivationFunctionType
    PSUM = bass.MemorySpace.PSUM

    nc = bacc.Bacc("TRN2", target_bir_lowering=False, debug=False)

    x_d = nc.dram_tensor("x", [BL, CIN, H, W], f32, kind="ExternalInput")
    # winograd weights, host-transformed: [ct, p(cin%128), j, ic, i, o(128)]
    uw_d = nc.dram_tensor("uw", [OC, 128, 4, IC, 4, 128], bf16, kind="ExternalInput")
    dwt_d = nc.dram_tensor("dwt", [LATENT, CIN], f32, kind="ExternalInput")    # [l,c]
    yt_d = nc.dram_tensor("yt", [LATENT, BL], f32, kind="ExternalInput")       # [l,b]
    db_d = nc.dram_tensor("db", [CIN, 1], f32, kind="ExternalInput")
    wsq_d = nc.dram_tensor("wsq", [IC, 128, COUT], f32, kind="ExternalInput")  # [ic,p,o]
    out_d = nc.dram_tensor("out", [BL, COUT, OH, OW], bf16, kind="ExternalOutput")

    with tile.TileContext(nc) as tc:
        with (
            tc.tile_pool(name="persist", bufs=1) as persist,
            tc.tile_pool(name="small", bufs=1) as small,
            tc.tile_pool(name="xstage", bufs=3) as xstage,
            tc.tile_pool(name="xeo", bufs=2) as xeo,
            tc.tile_pool(name="tpool", bufs=2) as tpool,
            tc.tile_pool(name="vpool", bufs=2) as vpool,
            tc.tile_pool(name="mpool", bufs=2) as mpool,
            tc.tile_pool(name="qpool", bufs=2) as qpool,
            tc.tile_pool(name="npool", bufs=2) as npool,
            tc.tile_pool(name="opool", bufs=2) as opool,
            tc.tile_pool(name="psq", bufs=2, space=PSUM) as psq,
        ):
            # ---- persistent winograd weights ----
            ubf = persist.tile([128, OC, 4, IC, 4, 128], bf16)

            # ---- param DMAs (style params first: they gate everything) ----
            dwt_sb = small.tile([128, 4, CIN], f32)
            nc.sync.dma_start(dwt_sb[:, :, :], dwt_d.ap().rearrange("(l p) c -> p l c", p=128))
            yt_sb = small.tile([128, 4, BL], f32)
            nc.sync.dma_start(yt_sb[:, :, :], yt_d.ap().rearrange("(l p) b -> p l b", p=128))
            db_sb = small.tile([128, 4, 1], f32)
            nc.sync.dma_start(db_sb[:, :, :], db_d.ap().rearrange("(c p) u -> p c u", p=128))
            wsq_sb = small.tile([128, IC, COUT], f32)
            for ic in range(IC):
                nc.sync.dma_start(wsq_sb[:, ic, :], wsq_d[ic])

            # ---- x (b0, g0) chunk 0 + U ct0, then the rest ----
            xq_first = []
            for ic in range(IC):
                xr = xstage.tile([128, 18, 64], f32, tag="xr")
                nc.sync.dma_start(xr[:, 0:18, :], x_d[0, ic * 128:(ic + 1) * 128, 0:18, :])
                xq_first.append(xr)
                if ic == 0:
                    nc.sync.dma_start(ubf[:, 0], uw_d[0])

            # ---- remaining U tiles ----
            for ct in range(1, OC):
                nc.sync.dma_start(ubf[:, ct], uw_d[ct])

            # ---- style: s = (y @ dense_w.T) * mul + b ; ssq = s^2 on ACT ----
            s_sb = small.tile([128, IC, BL], f32)
            ssq = small.tile([128, IC, BL], f32)
            for ctc in range(IC):
                ps = psq.tile([128, 4, 512], f32, tag="pq")
                for lc in range(4):
                    nc.tensor.matmul(
                        ps[:, 0, 0:BL],
                        dwt_sb[:, lc, ctc * 128:(ctc + 1) * 128],
                        yt_sb[:, lc, :],
                        start=(lc == 0),
                        stop=(lc == 3),
                    )
                nc.scalar.activation(
                    s_sb[:, ctc, :], ps[:, 0, 0:BL], AF.Identity,
                    bias=db_sb[:, ctc, :], scale=MUL_DENSE,
                )
                nc.scalar.activation(ssq[:, ctc, :], s_sb[:, ctc, :], AF.Square)

            # ---------------- transform emitters ----------------
            def emit_xdma(b, g, ic):
                r0, ntr = GROUPS[g]
                nrows = 2 * ntr + 2
                xr = xstage.tile([128, 18, 64], f32, tag="xr")
                nc.sync.dma_start(
                    xr[:, 0:nrows, :],
                    x_d[b, ic * 128:(ic + 1) * 128, 2 * r0:2 * r0 + nrows, :],
                )
                return xr

            def emit_eo(b, g, ic, xr):
                """style scale + even/odd column split + bf16 convert (ACT)."""
                r0, ntr = GROUPS[g]
                nrows = 2 * ntr + 2
                xe = xeo.tile([128, 2, 18, 32], bf16, tag="xe")
                nc.scalar.activation(
                    xe[:, 0, 0:nrows, :], xr[:, 0:nrows, 0:64:2], AF.Copy,
                    scale=s_sb[:, ic, b:b + 1])
                nc.scalar.activation(
                    xe[:, 1, 0:nrows, :], xr[:, 0:nrows, 1:64:2], AF.Copy,
                    scale=s_sb[:, ic, b:b + 1])
                return xe

            def emit_rowcol(b, g, ic, xe, vg, eng=None):
                """winograd row + col passes (bf16 stride-1 -> 2x mode).

                Row pass merges even/odd halves per combo; col pass merges the
                4 i-combos per j output. eng picks DVE (default) or GPSIMD."""
                eng = eng or nc.vector
                r0, ntr = GROUPS[g]
                n2 = 2 * ntr
                te = tpool.tile([128, 2, 4, 8, 32], bf16, tag="te")
                d = xe
                r_a = d[:, :, 0:n2:2, :]
                r_b = d[:, :, 1:n2 + 1:2, :]
                r_c = d[:, :, 2:n2 + 2:2, :]
                r_d = d[:, :, 3:n2 + 2:2, :]
                eng.tensor_sub(te[:, :, 0, 0:ntr, :], r_a, r_c)
                eng.tensor_add(te[:, :, 1, 0:ntr, :], r_b, r_c)
                eng.tensor_sub(te[:, :, 2, 0:ntr, :], r_c, r_b)
                eng.tensor_sub(te[:, :, 3, 0:ntr, :], r_b, r_d)
                tev = te[:, 0, :, 0:ntr, :]
                tov = te[:, 1, :, 0:ntr, :]
                c_a = tev[:, :, :, 0:31]
                c_b = tov[:, :, :, 0:31]
                c_c = tev[:, :, :, 1:32]
                c_d = tov[:, :, :, 1:32]
                eng.tensor_sub(vg[:, ic, 0, :, 0:ntr, :], c_a, c_c)
                eng.tensor_add(vg[:, ic, 1, :, 0:ntr, :], c_b, c_c)
                eng.tensor_sub(vg[:, ic, 2, :, 0:ntr, :], c_c, c_b)
                eng.tensor_sub(vg[:, ic, 3, :, 0:ntr, :], c_b, c_d)

            def new_vg():
                return vpool.tile([128, IC, 4, 4, 8, 31], bf16, tag="vg", name="vg")

            # ---- transforms for (b0, g0), plus a head start on (b0, g1):
            #      DVE is the steady-state constraint, so pre-pay its debt
            #      while the PE is still waiting on U/x DMAs ----
            vg_cur = new_vg()
            for ic in range(IC):
                xe0 = emit_eo(0, 0, ic, xq_first[ic])
                emit_rowcol(0, 0, ic, xe0, vg_cur)
            vg_g1 = new_vg()
            for ic in range(2):
                xr1 = emit_xdma(0, 1, ic)
                xe1 = emit_eo(0, 1, ic, xr1)
                emit_rowcol(0, 1, ic, xe1, vg_g1)

            # ---- demod: T = wsq.T @ ssq ; D = 1/sqrt(T + eps') ----
            #      (emitted after the transforms so the ACT sqrt does not
            #       stall the in-order ACT queue ahead of the eo copies)
            sqrt_t = small.tile([128, OC, BL], f32)
            d_sb = small.tile([128, OC, BL], f32)
            eps_sb = small.tile([128, 1], f32)
            nc.gpsimd.memset(eps_sb[:, :], EPS2)
            for ot in range(OC):
                ps = psq.tile([128, 4, 512], f32, tag="pq")
                for ic in range(IC):
                    nc.tensor.matmul(
                        ps[:, 0, 0:BL],
                        wsq_sb[:, ic, ot * 128:(ot + 1) * 128],
                        ssq[:, ic, :],
                        start=(ic == 0),
                        stop=(ic == 3),
                    )
                nc.scalar.activation(
                    sqrt_t[:, ot, :], ps[:, 0, 0:BL], AF.Sqrt,
                    bias=eps_sb[:, :], scale=1.0,
                )
                nc.vector.reciprocal(d_sb[:, ot, :], sqrt_t[:, ot, :])

            # ---------------- inverse-transform helpers ----------------
            def drain(pq, ms, j, N, dsc, on_dve):
                """PSUM quad -> bf16 staging, demod scale folded in."""
                if on_dve:
                    nc.vector.tensor_scalar_mul(ms[:, j, :, 0:N], pq[:, :, 0:N], dsc)
                else:
                    nc.scalar.activation(
                        ms[:, j, :, 0:N], pq[:, :, 0:N], AF.Copy, scale=dsc)

            def combine_ms(ms, nb, N, qeng):
                """A^T row-combine over all 4 j at once (bf16, 2x mode)."""
                qn = qpool.tile([128, 2, 4, 248], bf16, tag="qc")
                qeng.tensor_add(qn[:, 0, :, 0:N], ms[:, :, 0, 0:N], ms[:, :, 1, 0:N])
                qeng.tensor_sub(qn[:, 1, :, 0:N], ms[:, :, 1, 0:N], ms[:, :, 2, 0:N])
                nc.vector.tensor_add(nb[:, 0, :, 0:N], qn[:, 0, :, 0:N], ms[:, :, 2, 0:N])
                nc.vector.tensor_sub(nb[:, 1, :, 0:N], qn[:, 1, :, 0:N], ms[:, :, 3, 0:N])

            def stage2(b, g, ct, nb, qeng):
                """A col-combine; finals write the interleaved bf16 output."""
                r0, ntr = GROUPS[g]
                N = ntr * NTC
                ob = opool.tile([128, 8, 2, 31, 2], bf16, tag="ob")
                q2 = qpool.tile([128, 2, 2, 248], bf16, tag="q2")
                qeng.tensor_add(
                    q2[:, 0, :, 0:N], nb[:, :, 0, 0:N], nb[:, :, 1, 0:N])
                qeng.tensor_sub(
                    q2[:, 1, :, 0:N], nb[:, :, 1, 0:N], nb[:, :, 2, 0:N])
                nc.vector.tensor_add(
                    ob[:, 0:ntr, :, :, 0].transpose([0, 2, 1, 3]),
                    q2[:, 0, :, 0:N], nb[:, :, 2, 0:N])
                nc.vector.tensor_sub(
                    ob[:, 0:ntr, :, :, 1].transpose([0, 2, 1, 3]),
                    q2[:, 1, :, 0:N], nb[:, :, 3, 0:N])
                nc.sync.dma_start(
                    out_d[b, ct * 128:(ct + 1) * 128, 2 * r0:2 * r0 + 2 * ntr, :],
                    ob[:, 0:ntr, :, :, :],
                )

            # ---------------- main loop over (sample, row-group) ----------------
            for gi in range(2 * 4):
                b, g = divmod(gi, 4)
                r0, ntr = GROUPS[g]
                N = ntr * NTC
                nxt = gi + 1 if gi + 1 < 8 else None
                nb_, ng = divmod(nxt, 4) if nxt is not None else (None, None)
                if gi == 0:
                    vg_next = vg_g1
                elif nxt is not None:
                    vg_next = new_vg()
                else:
                    vg_next = None
                xq_next = [None] * IC
                xe_next = [None] * IC
                for ct in range(OC):
                    vct = (gi * 4 + ct) >= (32 - N_VCT)
                    nb = npool.tile([128, 2, 4, 248], bf16, tag="nb")
                    ms = mpool.tile([128, 4, 4, 248], bf16, tag="ms", name="ms")
                    for j in range(4):
                        pq = psq.tile([128, 4, 512], f32, tag="pq")
                        for i in range(4):
                            for ic in range(IC):
                                nc.tensor.matmul(
                                    pq[:, i, 0:N],
                                    ubf[:, ct, j, ic, i, :],
                                    vg_cur[:, ic, j, i, 0:ntr, :],
                                    start=(ic == 0),
                                    stop=(ic == IC - 1),
                                )
                        drain(pq, ms, j, N, d_sb[:, ct, b:b + 1], vct)
                    qeng = nc.gpsimd if (ct % 4) < GPS_Q else nc.vector
                    combine_ms(ms, nb, N, qeng)
                    # prefetch next group's input pipeline (group 0: chunks 0,1
                    # of g1 were already done in the prologue)
                    if nxt is not None:
                        lo = 2 if gi == 0 else 0
                        if ct == 0:
                            for c in range(lo, 3):
                                xq_next[c] = emit_xdma(nb_, ng, c)
                            for c in range(lo, 2):
                                xe_next[c] = emit_eo(nb_, ng, c, xq_next[c])
                            if lo == 0:
                                emit_rowcol(nb_, ng, 0, xe_next[0], vg_next)
                        elif ct == 1:
                            xq_next[3] = emit_xdma(nb_, ng, 3)
                            xe_next[2] = emit_eo(nb_, ng, 2, xq_next[2])
                            if lo == 0:
                                emit_rowcol(nb_, ng, 1, xe_next[1], vg_next)
                        elif ct == 2:
                            xe_next[3] = emit_eo(nb_, ng, 3, xq_next[3])
                            emit_rowcol(nb_, ng, 2, xe_next[2], vg_next)
                        else:
                            emit_rowcol(nb_, ng, 3, xe_next[3], vg_next)
                    stage2(b, g, ct, nb, qeng)
                vg_cur = vg_next

    nc.compile()
    return nc


def run(inputs, profile=False):
    """inputs: dict with full-size x/y/dense_w/dense_b/weight numpy arrays.
    Returns (out [16,512,62,62] float32, exec_time_ns or None)."""
    import ml_dtypes
    from concourse.bass_utils import run_bass_kernel_spmd

    if "nc" not in _cache:
        _cache["nc"] = build()
    nc = _cache["nc"]

    x = np.ascontiguousarray(np.asarray(inputs["x"], dtype=np.float32))
    y = np.ascontiguousarray(np.asarray(inputs["y"], dtype=np.float32))
    dense_w = np.asarray(inputs["dense_w"], dtype=np.float32)
    dense_b = np.asarray(inputs["dense_b"], dtype=np.float32)
    weight = np.asarray(inputs["weight"], dtype=np.float32)

    # host-side winograd weight transform U = G w G^T -> [ct, p, j, ic, i, o] bf16
    G = np.array([[1, 0, 0], [.5, .5, .5], [.5, -.5, .5], [0, 0, 1]], np.float32)
    U = np.einsum("ak,oikl,bl->oiab", G, weight, G, optimize=True)  # [o, c, a, b]
    uw = np.ascontiguousarray(
        U.reshape(OC, 128, IC, 128, 4, 4).transpose(0, 3, 5, 2, 4, 1)
        .astype(ml_dtypes.bfloat16))
    # wsq[o,c] = sum_k w^2 (fp32, host) -> [ic, p, o]
    wsq = np.ascontiguousarray(
        (weight.astype(np.float64) ** 2).sum(axis=(2, 3)).astype(np.float32)
        .T.reshape(IC, 128, COUT))
    dwt = np.ascontiguousarray(dense_w.T)                          # [l, c]
    db = np.ascontiguousarray(dense_b.reshape(CIN, 1))

    in_maps = []
    for c in range(N_CORES):
        sl = slice(c * BL, (c + 1) * BL)
        in_maps.append({
            "x": x[sl],
            "uw": uw,
            "dwt": dwt,
            "yt": np.ascontiguousarray(y[sl].T),                   # [l, b]
            "db": db,
            "wsq": wsq,
        })

    if profile:
        _ensure_ntff_hook()
    res = run_bass_kernel_spmd(
        nc, in_maps, core_ids=list(range(N_CORES)), trace=profile)
    out = np.concatenate(
        [np.asarray(r["out"]).astype(np.float32) for r in res.results], axis=0)
    return out, res.exec_time_ns


def kernel(**inputs) -> np.ndarray:
    out, _ = run(inputs)
    return out
